# revision 37
# baseline (speedup 1.0000x reference)
"""Trainium2 Bass kernel for CompanyIndustryAttention (gnn_message_passing).

Strategy (all 8 cores, zero collectives):
  - Companies sharded into 8 contiguous ranges of 2500 rows; each edge is
    owned by the core that owns its src company, so the segment-sum scatter
    is core-local (no all-reduce needed).
  - K/V side: tgt indexes only 500 industries, so softmax over the full
    edge set collapses to a count-weighted softmax over the 500 industries:
        sum_k exp(s_tgt[k]) v_tgt[k] = sum_u cnt_u exp(s_u) v_u
    The count weighting is applied as a per-partition (per-industry) bias
    ln(cnt_u) on the Exp activation (exp(s + ln c) = c * exp(s)), keeping
    it in fp32.  This turns O(E x E) attention into O(E x 500).
  - Device work is fully dense/static: host does index-only preprocessing
    (sort edges by src, pack into per-company-tile slot windows, gather
    company_x rows for the Q side, count edges).  The compiled program is
    identical on all cores; per-core differences live in the input tensors.
  - Segment-sum on device = one-hot(src) matmuls on the tensor engine over
    a fixed 2-e-tile window per company tile (host packing guarantees the
    window); layernorm tail runs node-major with bn_stats/bn_aggr.

Performance notes:
  - Matmul operands are bf16 (1 PE cycle/column vs 4 for fp32, one HW
    instruction instead of two, half the LDWEIGHTS time); accumulation
    stays fp32 in PSUM, and the layernorm/softmax-denominator paths stay
    fp32.
  - v-bias is folded into the output bias host-side
    ((ctx+bv) @ Wo.T + bo == ctx @ Wo.T + (bo + bv @ Wo.T)).
  - Softmax denominators for all 4 heads are normalized once per chunk:
    one reciprocal_approx_fast + two 4x128 mask matmuls broadcast the
    per-edge reciprocals to the head rows.
  - Elementwise adds/one-hots/layernorm affine run on the otherwise-idle
    Pool (gpsimd) engine.
  - Attention runs chunk-outer so attn_out and the segment-sum/layernorm
    tail of early company tiles overlap with attention of later chunks.
"""

import os
import sys

import numpy as np

for _p in ("/opt/trn_rl_repo",):
    if _p not in sys.path and os.path.isdir(_p):
        sys.path.insert(0, _p)

import ml_dtypes

import concourse.bass as bass
import concourse.bacc as bacc
import concourse.tile as tile
from concourse import mybir
from concourse.bass_utils import run_bass_kernel_spmd

F32 = mybir.dt.float32
BF16 = mybir.dt.bfloat16
NPBF = ml_dtypes.bfloat16
AF = mybir.ActivationFunctionType
ALU = mybir.AluOpType

# Problem shapes (hardcoded per the spec).
N_COMPANY, N_INDUSTRY, E = 20000, 500, 8192
CC, CI, D, H = 256, 128, 256, 4
HD = D // H  # 64
SCALE = 1.0 / float(np.sqrt(np.float32(HD)))

NCORES = 8
NSH = N_COMPANY // NCORES       # 2500 companies per core
NCT = 20                        # company tiles (19 x 128 + 68)
E_CAP = 1280                    # padded edge slots per core (10 e-tiles)
NET = E_CAP // 128              # 10 edge tiles
SLOTS = E_CAP // NCT            # 64 slots per company tile
E_CHUNKS = [(0, 512), (512, 1024), (1024, 1280)]
# company tiles whose e-tile window is complete after chunk ci's attn_out
READY_J = [range(0, 6), range(6, 14), range(14, 20)]

_CACHE = {}
TRACE = False        # set by test.py to request an NTFF profile
LAST_RESULT = None   # BassKernelResults of the most recent run


def _csz(j):
    return min(128, NSH - 128 * j)


def _window(j):
    return [t for t in (j // 2, j // 2 + 1) if t < NET]


def build_program():
    nc = bacc.Bacc(debug=False)

    def din(name, shape, dt=F32):
        return nc.declare_dram_parameter(name, list(shape), dt, isOutput=False)

    cxT = din("cxT", (CC, NSH), BF16)        # company_x shard, transposed
    qxT = din("qxT", (CC, E_CAP), BF16)      # company_x rows per edge slot
    ixT = din("ixT", (CI, N_INDUSTRY), BF16)  # industry_x transposed
    WcT = din("WcT", (CC, D), BF16)
    WiT = din("WiT", (CI, D), BF16)
    wqT = din("wqT", (D, D), BF16)           # (wq*scale).T
    wkT = din("wkT", (D, D), BF16)
    wvT = din("wvT", (D, D), BF16)
    woT = din("woT", (D, D), BF16)           # w_out.T
    bc = din("bc", (1, D))
    bcbo = din("bcbo", (2, D), BF16)         # rows: bc, bo (bias matmul rhs)
    bi = din("bi", (1, D))
    bq = din("bq", (1, D))                   # bq*scale
    bk = din("bk", (1, D))
    gamma16 = din("gamma16", (1, D), BF16)
    beta16 = din("beta16", (1, D), BF16)
    lncnt = din("lncnt", (512,))             # ln(edge count per industry)
    rslot = din("rslot", (E_CAP,))           # 1/(cnt[src[slot]]+1e-6), 0=pad
    # one-hot scatter blocks: rows 256*j+128*wi map slot partitions of
    # window tile wi to companies 128j..128j+127
    ohm = din("ohm", (NCT * 2 * 128, 128), BF16)
    # bias matmul lhsT per company tile: row0 = ones (bc), row1 = cnt*recip
    # per company (bo scale)
    crm = din("crm", (NCT * 2, 128), BF16)
    out = nc.declare_dram_parameter("out", [NSH, D], BF16, isOutput=True)

    def wrap_ap(t, cols):
        # [n] DRAM -> [128, cols] SBUF with element (p + 128*c) at [p, c]
        return bass.AP(tensor=t[:].tensor, offset=0, ap=[[1, 128], [128, cols]])

    with tile.TileContext(nc) as tc:
        with (
            tc.tile_pool(name="const", bufs=1) as const,
            tc.tile_pool(name="persist", bufs=1) as persist,
            tc.tile_pool(name="work", bufs=3) as work,
            # PSUM (8 banks x 2KB/partition): ps x3 + pc x2 + pb + pagg x2
            tc.tile_pool(name="psp", bufs=3, space="PSUM") as psp,
            tc.tile_pool(name="pcp", bufs=2, space="PSUM") as pcp,
            tc.tile_pool(name="pmp", bufs=1, space="PSUM") as pmp,
        ):
            dma = nc.sync.dma_start
            mm = nc.tensor.matmul

            # ---------------- constants / params into SBUF ----------------
            def load2(t, rows, cols):
                tiles = []
                for k in range(rows // 128):
                    s = const.tile([128, cols], BF16, name=f"w_{t.name}_{k}",
                                   tag=f"w_{t.name}_{k}")
                    dma(out=s, in_=t[k * 128:(k + 1) * 128, :])
                    tiles.append(s)
                return tiles

            WcT_sb = load2(WcT, CC, D)
            WiT_sb = load2(WiT, CI, D)
            wqT_sb = load2(wqT, D, D)
            wkT_sb = load2(wkT, D, D)
            wvT_sb = load2(wvT, D, D)
            woT_sb = load2(woT, D, D)
            ixT_sb = load2(ixT, CI, N_INDUSTRY)
            qxT_sb = load2(qxT, CC, E_CAP)
            cxT_sb = load2(cxT, CC, NSH)

            def bcast_row(t, tag):
                s = const.tile([128, D], BF16, tag=tag)
                dma(out=s, in_=t[:, :].to_broadcast([128, D]))
                return s

            gam_b = bcast_row(gamma16, "gam_b")
            bet_b = bcast_row(beta16, "bet_b")

            def col_pp(t, tag):
                # [1, 256] DRAM -> [128, 2] SBUF per-partition columns
                s = const.tile([128, 2], F32, tag=tag)
                dma(out=s, in_=bass.AP(tensor=t[:, :].tensor, offset=0,
                                       ap=[[1, 128], [128, 2]]))
                return s

            bc_pp = col_pp(bc, "bc_pp")
            bi_pp = col_pp(bi, "bi_pp")
            bq_pp = col_pp(bq, "bq_pp")
            bk_pp = col_pp(bk, "bk_pp")

            # one-hot scatter tiles, host-precomputed
            oh_sb = {}
            for j in range(NCT):
                for wi, t in enumerate(_window(j)):
                    s = const.tile([128, 128], BF16, name=f"oh{j}_{wi}",
                                   tag=f"oh{j}_{wi}")
                    r0 = 256 * j + 128 * wi
                    dma(out=s, in_=ohm[r0:r0 + 128, :])
                    oh_sb[(j, wi)] = s

            rsl_sb = const.tile([128, NET], F32, name="rsl_sb", tag="rsl_sb")
            dma(out=rsl_sb, in_=wrap_ap(rslot, NET))
            lnc_pp = const.tile([128, 4], F32, name="lnc_pp", tag="lnc_pp")
            dma(out=lnc_pp, in_=wrap_ap(lncnt, 4))

            crm_sb = []
            for j in range(NCT):
                s = const.tile([2, 128], BF16, name=f"crm{j}", tag=f"crm{j}")
                dma(out=s, in_=crm[2 * j:2 * j + 2, :])
                crm_sb.append(s)
            bcbo_sb = const.tile([2, D], BF16, name="bcbo_sb", tag="bcbo_sb")
            dma(out=bcbo_sb, in_=bcbo[:, :])

            ones64 = const.tile([1, HD], F32, name="ones64", tag="ones64")
            nc.gpsimd.memset(ones64, 1.0)
            eps_sb = const.tile([128, 1], F32, name="eps_sb", tag="eps_sb")
            nc.gpsimd.memset(eps_sb, 1e-5)

            def ppbias(colsb, h):
                # per-partition bias [64,1] for head h from a [128,2] column tile
                return colsb[64 * (h % 2):64 * (h % 2) + 64, h // 2:h // 2 + 1]

            # ---------------- industry side: ihT, kh', v' -------------------
            # industry_hT [D, 500] feature-major
            ihT = [persist.tile([128, N_INDUSTRY], BF16, name=f"ihT{d}", tag=f"ihT{d}")
                   for d in range(2)]
            for dti in range(2):
                ps = psp.tile([128, 512], F32, name="ps", tag="ps")
                mm(ps[:, 0:N_INDUSTRY],
                   WiT_sb[0][:, dti * 128:(dti + 1) * 128], ixT_sb[0],
                   start=True, stop=True)
                nc.scalar.activation(ihT[dti], ps[:, 0:N_INDUSTRY], AF.Identity,
                                     bias=bi_pp[:, dti:dti + 1], scale=1.0)

            # kh' per head: [64, 500]
            khp = [persist.tile([128, N_INDUSTRY], BF16, name=f"khp{h}", tag=f"khp{h}")
                   for h in range(H)]
            for h in range(H):
                ps = psp.tile([128, 512], F32, name="ps", tag="ps")
                for k in range(2):
                    mm(ps[0:64, 0:N_INDUSTRY],
                       wkT_sb[k][:, h * 64:(h + 1) * 64], ihT[k],
                       start=(k == 0), stop=(k == 1))
                nc.scalar.activation(khp[h][0:64, :], ps[0:64, 0:N_INDUSTRY],
                                     AF.Identity,
                                     bias=ppbias(bk_pp, h), scale=1.0)

            # v' node-major [500-part, H, 65]; col 64 of each head = 1.0
            # (no +bv: folded into bo host-side)
            usz = [128, 128, 128, 116]
            vp = [persist.tile([128, H, HD + 1], BF16, name=f"vp{t}", tag=f"vp{t}")
                  for t in range(4)]
            for t in range(4):
                u0, u1 = t * 128, t * 128 + usz[t]
                ps = psp.tile([128, 512], F32, name="ps", tag="ps")
                for k in range(2):
                    mm(ps[0:usz[t], 0:D], ihT[k][:, u0:u1], wvT_sb[k],
                       start=(k == 0), stop=(k == 1))
                for h in range(H):
                    nc.scalar.activation(vp[t][0:usz[t], h, 0:HD],
                                         ps[0:usz[t], h * 64:(h + 1) * 64],
                                         AF.Copy)
                nc.gpsimd.memset(vp[t][:, :, HD:HD + 1], 1.0)

            # ---------------- q side: q_h then qh' --------------------------
            # q_hT [D, E_CAP] = Wc @ qxT + bc   (feature-major)
            qhT = [persist.tile([128, E_CAP], BF16, name=f"qhT{d}", tag=f"qhT{d}")
                   for d in range(2)]
            for dti in range(2):
                for c0, c1 in E_CHUNKS:
                    ps = psp.tile([128, 512], F32, name="ps", tag="ps")
                    for k in range(2):
                        mm(ps[:, 0:c1 - c0],
                           WcT_sb[k][:, dti * 128:(dti + 1) * 128],
                           qxT_sb[k][:, c0:c1],
                           start=(k == 0), stop=(k == 1))
                    nc.scalar.activation(qhT[dti][:, c0:c1], ps[:, 0:c1 - c0],
                                         AF.Identity,
                                         bias=bc_pp[:, dti:dti + 1], scale=1.0)

            # qh' per head [64, E_CAP] (scaled)
            qhp = [persist.tile([128, E_CAP], BF16, name=f"qhp{h}", tag=f"qhp{h}")
                   for h in range(H)]
            for h in range(H):
                for c0, c1 in E_CHUNKS:
                    ps = psp.tile([128, 512], F32, name="ps", tag="ps")
                    for k in range(2):
                        mm(ps[0:64, 0:c1 - c0],
                           wqT_sb[k][:, h * 64:(h + 1) * 64],
                           qhT[k][:, c0:c1],
                           start=(k == 0), stop=(k == 1))
                    nc.scalar.activation(qhp[h][0:64, c0:c1],
                                         ps[0:64, 0:c1 - c0], AF.Identity,
                                         bias=ppbias(bq_pp, h), scale=1.0)

            # ---------- attention + attn_out + segsum/LN, chunk-outer -------
            ctxT = [persist.tile([128, E_CAP], BF16, name=f"ctxT{d}", tag=f"ctxT{d}")
                    for d in range(2)]
            # aor[t] = (ctx @ w_out.T + bo) * rslot  (per-slot recip-scaled)
            aor = [persist.tile([128, D], BF16, name=f"aor{t}", tag=f"aor{t}")
                   for t in range(NET)]

            def tail(j):
                # x = onehot.T@aor + company_h + bc accumulated in one PSUM
                # group, then layernorm straight off PSUM.
                cs = _csz(j)
                pagg = pmp.tile([128, D], F32, name="pagg", tag="pagg",
                                bufs=2)
                win = _window(j)
                for wi, t in enumerate(win):
                    mm(pagg[0:cs, :], oh_sb[(j, wi)][:, 0:cs], aor[t],
                       start=(wi == 0), stop=False)
                for k in range(2):
                    mm(pagg[0:cs, :], cxT_sb[k][:, 128 * j:128 * j + cs],
                       WcT_sb[k], start=False, stop=False)
                mm(pagg[0:cs, :], crm_sb[j][:, 0:cs], bcbo_sb,
                   start=False, stop=True)
                # layernorm along free axis, stats off PSUM.
                # rstd = exp(-0.5*ln(var+eps)): ln/exp share one act table
                # with the attention Exp, avoiding table reloads (sqrt does
                # not).
                st = work.tile([128, nc.vector.BN_STATS_DIM], F32, name="st", tag="st")
                nc.vector.bn_stats(out=st[0:cs, :], in_=pagg[0:cs, :])
                mv = work.tile([128, nc.vector.BN_AGGR_DIM], F32, name="mv", tag="mv")
                nc.vector.bn_aggr(out=mv[0:cs, :], in_=st[0:cs, :])
                lv = work.tile([128, 1], F32, name="lv", tag="lv")
                nc.scalar.activation(lv[0:cs, :], mv[0:cs, 1:2], AF.Ln,
                                     bias=eps_sb[0:cs, :], scale=1.0)
                rstd = work.tile([128, 1], F32, name="rstd", tag="rstd")
                nc.scalar.activation(rstd[0:cs, :], lv[0:cs, :], AF.Exp,
                                     scale=-0.5)
                xn = work.tile([128, D], BF16, name="xn", tag="xn")
                nc.vector.tensor_scalar(
                    out=xn[0:cs, :], in0=pagg[0:cs, :],
                    scalar1=mv[0:cs, 0:1], scalar2=rstd[0:cs, :],
                    op0=ALU.subtract, op1=ALU.mult)
                y = work.tile([128, D], BF16, name="y", tag="y")
                nc.vector.tensor_tensor(out=y[0:cs, :], in0=xn[0:cs, :],
                                        in1=gam_b[0:cs, :], op=ALU.mult)
                nc.vector.tensor_tensor(out=y[0:cs, :], in0=y[0:cs, :],
                                        in1=bet_b[0:cs, :], op=ALU.add)
                dma(out=out[128 * j:128 * j + cs, :], in_=y[0:cs, :])

            for ci, (c0, c1) in enumerate(E_CHUNKS):
                cw = c1 - c0
                for h in range(H):
                    pc = pcp.tile([128, 512], F32, name="pc", tag="pc")
                    for t in range(4):
                        u0, u1 = t * 128, t * 128 + usz[t]
                        ps = psp.tile([128, 512], F32, name="ps", tag="ps")
                        mm(ps[0:usz[t], 0:cw],
                           khp[h][0:64, u0:u1], qhp[h][0:64, c0:c1],
                           start=True, stop=True)
                        pexp = work.tile([128, 512], BF16, name="pexp", tag="pexp")
                        nc.scalar.activation(pexp[0:usz[t], 0:cw],
                                             ps[0:usz[t], 0:cw], AF.Exp,
                                             bias=lnc_pp[0:usz[t], t:t + 1],
                                             scale=1.0)
                        mm(pc[0:65, 0:cw], vp[t][0:usz[t], h, :],
                           pexp[0:usz[t], 0:cw],
                           start=(t == 0), stop=(t == 3))
                    # normalize rows 0:64 by row 64 (count-weighted softmax).
                    # (reciprocal_approx_fast misreads PSUM at base partition
                    # 64 on HW; the table reciprocal handles it.)
                    rd = work.tile([1, 512], F32, name="rd", tag="rd")
                    nc.vector.reciprocal(rd[:, 0:cw], pc[64:65, 0:cw])
                    pb = pmp.tile([128, 512], F32, name="pb", tag="pb")
                    mm(pb[0:64, 0:cw], ones64, rd[:, 0:cw],
                       start=True, stop=True)
                    rb = work.tile([128, 512], F32, name="rb", tag="rb")
                    nc.vector.tensor_copy(out=rb[0:64, 0:cw],
                                          in_=pb[0:64, 0:cw])
                    nc.vector.tensor_tensor(
                        out=ctxT[h // 2][64 * (h % 2):64 * (h % 2) + 64, c0:c1],
                        in0=pc[0:64, 0:cw], in1=rb[0:64, 0:cw], op=ALU.mult)

                # attn_out for the e-tiles completed by this chunk,
                # pre-scaled by the per-slot segment reciprocal (the bo bias
                # rides the tail bias matmul instead)
                for t in range(c0 // 128, c1 // 128):
                    ps = psp.tile([128, 512], F32, name="ps", tag="ps")
                    for k in range(2):
                        mm(ps[:, 0:D], ctxT[k][:, t * 128:(t + 1) * 128],
                           woT_sb[k], start=(k == 0), stop=(k == 1))
                    nc.vector.tensor_scalar_mul(out=aor[t], in0=ps[:, 0:D],
                                                scalar1=rsl_sb[:, t:t + 1])

                for j in READY_J[ci]:
                    tail(j)

    if not nc.is_finalized():
        nc.finalize()   # Bacc: runs wait-splitting etc. to meet HW limits
    return nc


def host_inputs(company_x, industry_x, edge_index, Wc, bc, Wi, bi,
                w_in, b_in, w_out, b_out, gamma, beta):
    """Shared (replicated) device tensors from the full problem inputs."""
    wq, wk, wv = np.split(w_in, 3, axis=0)
    bq, bk, bv = np.split(b_in, 3)
    tgt = edge_index[1].astype(np.int64)
    tgt_cnt = np.bincount(tgt, minlength=N_INDUSTRY).astype(np.float32)
    with np.errstate(divide="ignore"):
        lncnt = np.log(tgt_cnt).astype(np.float32)
    lncnt = np.where(tgt_cnt > 0, lncnt, np.float32(-1e30))
    lnc512 = np.full(512, -1e30, np.float32)
    lnc512[:N_INDUSTRY] = lncnt
    bo2 = (b_out + bv @ w_out.T).astype(np.float32)
    bf = lambda a: np.ascontiguousarray(a).astype(NPBF)
    return {
        "ixT": bf(industry_x.T),
        "WcT": bf(Wc.T),
        "WiT": bf(Wi.T),
        "wqT": bf((wq * np.float32(SCALE)).T),
        "wkT": bf(wk.T),
        "wvT": bf(wv.T),
        "woT": bf(w_out.T),
        "bc": bc.reshape(1, D),
        "bcbo": bf(np.stack([bc, bo2])),
        "bi": bi.reshape(1, D),
        "bq": (bq * np.float32(SCALE)).reshape(1, D),
        "bk": bk.reshape(1, D),
        "gamma16": bf(gamma.reshape(1, D)),
        "beta16": bf(beta.reshape(1, D)),
        "lncnt": lnc512,
    }


def _prep_core(core, company_x, edge_index, tgt_cnt):
    """Host-side index preprocessing for one core. Returns per-core arrays."""
    src = edge_index[0].astype(np.int64)
    lo = core * NSH
    sel = np.nonzero((src >= lo) & (src < lo + NSH))[0]
    ls = src[sel] - lo
    order = np.argsort(ls, kind="stable")
    ls = ls[order]

    ctile = (ls // 128).astype(np.int64)
    cnts = np.bincount(ctile, minlength=NCT)

    slot_of = np.empty(len(ls), dtype=np.int64)
    s = 0
    pos = 0
    for j in range(NCT):
        s = max(SLOTS * j, s)
        e = s + cnts[j]
        if cnts[j] > 0:
            lo_t, hi_t = s // 128, (e - 1) // 128
            if not ({lo_t, hi_t} <= set(_window(j))) or e > E_CAP:
                return None  # packing violated -> caller falls back
            slot_of[pos:pos + cnts[j]] = np.arange(s, e)
            pos += cnts[j]
        s = e

    srci = np.full(E_CAP, -1, dtype=np.int64)
    srci[slot_of] = ls
    qx = np.broadcast_to(company_x[lo], (E_CAP, CC)).copy()
    qx[slot_of] = company_x[lo + ls]

    ccnt = np.bincount(ls, minlength=NSH).astype(np.float32)
    recip = np.float32(1.0) / (ccnt + np.float32(1e-6))
    rslot = np.zeros(E_CAP, dtype=np.float32)
    rslot[slot_of] = recip[ls]

    ohm = np.zeros((NCT * 2 * 128, 128), dtype=NPBF)
    cols = np.arange(128)
    for j in range(NCT):
        for wi, t in enumerate(_window(j)):
            blk = (srci[128 * t:128 * (t + 1), None] == (128 * j + cols)[None, :])
            ohm[256 * j + 128 * wi:256 * j + 128 * (wi + 1), :] = blk

    # bias matmul lhsT rows: ones (bc applies to every company) and
    # cnt*recip (bo scale: bo summed over edges, divided by cnt+eps)
    crm = np.zeros((NCT * 2, 128), dtype=np.float32)
    crf = (ccnt * recip).astype(np.float32)
    for j in range(NCT):
        cs = min(128, NSH - 128 * j)
        crm[2 * j, :] = 1.0
        crm[2 * j + 1, 0:cs] = crf[128 * j:128 * j + cs]

    return {
        "cxT": np.ascontiguousarray(company_x[lo:lo + NSH].T).astype(NPBF),
        "qxT": np.ascontiguousarray(qx.T).astype(NPBF),
        "ohm": ohm,
        "crm": crm.astype(NPBF),
        "rslot": rslot,
    }


def _numpy_fallback(company_x, industry_x, edge_index, Wc, bc, Wi, bi,
                    w_in, b_in, w_out, b_out, gamma, beta):
    # Correctness safety net for inputs whose edge distribution breaks the
    # compiled packing assumptions. Mirrors the reference computation.
    company_h = company_x @ Wc.T + bc
    industry_h = industry_x @ Wi.T + bi
    src, tgt = edge_index[0], edge_index[1]
    e = src.shape[0]
    wq, wk, wv = np.split(w_in, 3, axis=0)
    bq, bk, bv = np.split(b_in, 3)
    qh = (company_h[src] @ wq.T + bq).reshape(e, H, HD)
    kh = (industry_h[tgt] @ wk.T + bk).reshape(e, H, HD)
    vh = (industry_h[tgt] @ wv.T + bv).reshape(e, H, HD)
    scores = np.einsum("qhd,khd->hqk", qh / np.sqrt(HD), kh)
    scores -= scores.max(-1, keepdims=True)
    p = np.exp(scores)
    attn = p / p.sum(-1, keepdims=True)
    ctx = np.einsum("hqk,khd->qhd", attn, vh).reshape(e, D)
    attn_out = ctx @ w_out.T + b_out
    agg = np.zeros((N_COMPANY, D), np.float32)
    np.add.at(agg, src, attn_out)
    counts = np.bincount(src, minlength=N_COMPANY).astype(np.float32)
    pooled = agg / (counts[:, None] + 1e-6)
    out = company_h + pooled
    mean = out.mean(-1, keepdims=True)
    var = out.var(-1, keepdims=True)
    return ((out - mean) / np.sqrt(var + 1e-5) * gamma + beta).astype(np.float32)


def kernel(company_x, industry_x, edge_index, Wc, bc, Wi, bi,
           w_in, b_in, w_out, b_out, gamma, beta):
    company_x = np.asarray(company_x, dtype=np.float32)
    industry_x = np.asarray(industry_x, dtype=np.float32)
    edge_index = np.asarray(edge_index)
    Wc = np.asarray(Wc, np.float32); bc = np.asarray(bc, np.float32)
    Wi = np.asarray(Wi, np.float32); bi = np.asarray(bi, np.float32)
    w_in = np.asarray(w_in, np.float32); b_in = np.asarray(b_in, np.float32)
    w_out = np.asarray(w_out, np.float32); b_out = np.asarray(b_out, np.float32)
    gamma = np.asarray(gamma, np.float32); beta = np.asarray(beta, np.float32)

    tgt = edge_index[1].astype(np.int64)
    tgt_cnt = np.bincount(tgt, minlength=N_INDUSTRY).astype(np.float32)

    cores = []
    for core in range(NCORES):
        pc = _prep_core(core, company_x, edge_index, tgt_cnt)
        if pc is None:
            print("kernel.py: edge packing fell outside compiled windows; "
                  "using host fallback", file=sys.stderr)
            return _numpy_fallback(company_x, industry_x, edge_index, Wc, bc,
                                   Wi, bi, w_in, b_in, w_out, b_out,
                                   gamma, beta)
        cores.append(pc)

    shared = host_inputs(company_x, industry_x, edge_index, Wc, bc, Wi, bi,
                         w_in, b_in, w_out, b_out, gamma, beta)

    if "nc" not in _CACHE:
        _CACHE["nc"] = build_program()
    nc = _CACHE["nc"]

    in_maps = [{**shared, **cores[i]} for i in range(NCORES)]
    kw = {}
    if TRACE:
        kw = {"trace": True, "tmpdir": os.environ.get("BASS_TRACE_DIR")}
    res = run_bass_kernel_spmd(nc, in_maps, list(range(NCORES)), **kw)
    global LAST_RESULT
    LAST_RESULT = res
    return np.concatenate(
        [np.asarray(res.results[i]["out"], np.float32) for i in range(NCORES)],
        axis=0)
